# revision 10
# baseline (speedup 1.0000x reference)
"""CurrentHistoricalAttention Trainium2 kernel (8 NeuronCores, data-parallel over B).

Math (per batch row b):
    q = [h_tilde; c_t] @ Waq.T + ba          [H]
    score = tanh(q[None, :] + h_history[b] @ Wah.T)   [D, H]
    au = score @ v_t                          [D]
    alpha = softmax(au)                       [D]
    e_t = alpha @ h_history[b]                [H]
Returns (e_t [B, H], alpha [B, D]).

Design (per core, 128 rows of B):
  - h_history tiles are loaded once, as f32r (rounded fp32, 1 cyc/row on PE),
    in natural [d, h] layout; PE-transposed on-chip to [h, d] for the
    score projection (contraction over h); the natural copy feeds the final
    alpha-weighted sum (contraction over d).
  - score projection: lhsT = Wah.T chunks (stationary), rhs = transposed
    h_history, accumulate over 4 k-chunks in PSUM.
  - tanh + (q+ba) bias fused into one ScalarE activation per [128, 256] tile.
  - v-dot as M=1 f32r matmuls; softmax batched over groups of 16 rows;
    alpha transposed via PE for the weighted-sum matmuls.
"""
import numpy as np
from contextlib import ExitStack

H = 512
B = 1024
D = 256
NCORES = 8
BS = B // NCORES        # 128 rows per core
GROUP = 16              # rows per softmax group
PPG = GROUP // 2        # pairs per group
NPAIR = BS // 2         # 64 pairs per core
NAT_BUFS = 12

import concourse.bass as bass
import concourse.mybir as mybir
import concourse.tile as tile
from concourse import bacc
from concourse.bass_utils import run_bass_kernel_spmd
from concourse.masks import make_identity

F32 = mybir.dt.float32
F32R = mybir.dt.float32r
AF = mybir.ActivationFunctionType
AX = mybir.AxisListType

_NC_CACHE = {}


def build_nc():
    nc = bacc.Bacc("TRN2", target_bir_lowering=False, debug=False)

    ht_d = nc.dram_tensor("h_tilde", [BS, H], F32, kind="ExternalInput").ap()
    ct_d = nc.dram_tensor("c_t", [BS, H], F32, kind="ExternalInput").ap()
    hh_d = nc.dram_tensor("h_history", [BS, D, H], F32, kind="ExternalInput").ap()
    waqT_d = nc.dram_tensor("WaqT", [2 * H, H], F32, kind="ExternalInput").ap()
    wahT_d = nc.dram_tensor("WahT", [H, H], F32, kind="ExternalInput").ap()
    ba_d = nc.dram_tensor("ba", [H], F32, kind="ExternalInput").ap()
    v_d = nc.dram_tensor("v_t", [H], F32, kind="ExternalInput").ap()

    e_out = nc.dram_tensor("e_t", [BS, H], F32, kind="ExternalOutput").ap()
    alpha_out = nc.dram_tensor("alpha", [BS, D], F32, kind="ExternalOutput").ap()
    au_stage = nc.dram_tensor("au_stage", [NPAIR, 2, 256], F32, kind="Internal").ap()
    e_view = e_out.rearrange("(j a) d -> j a d", a=2)

    with tile.TileContext(nc) as tc, ExitStack() as ctx:
        # ---- persistent constants ----
        const = ctx.enter_context(tc.tile_pool(name="const", bufs=1))
        ident = const.tile([128, 128], F32)
        make_identity(nc, ident[:])
        ident_r = const.tile([128, 128], F32R)
        nc.vector.tensor_copy(ident_r[:], ident[:])

        wahT_sb = const.tile([128, 4, H], F32R)     # [h%128, h//128, g]
        nc.gpsimd.dma_start(
            wahT_sb[:], wahT_d.rearrange("(kc p) g -> p kc g", p=128)
        )
        v_sb = const.tile([128, 4], F32)
        nc.sync.dma_start(v_sb[:], v_d.rearrange("(k p) -> p k", p=128))
        v_r = const.tile([128, 4], F32R)
        nc.vector.tensor_copy(v_r[:], v_sb[:])
        ba_sb = const.tile([128, 4], F32)
        nc.sync.dma_start(ba_sb[:], ba_d.rearrange("(m p) -> p m", p=128))

        qb_sb = const.tile([128, 4, BS], F32)        # [g%128, g//128, b]

        # ---- preamble: q projection (transposed) ----
        with ExitStack() as pctx:
            pre = pctx.enter_context(tc.tile_pool(name="pre", bufs=1))
            pre_ps = pctx.enter_context(tc.tile_pool(name="pre_ps", bufs=2, space="PSUM"))

            waqT_sb = pre.tile([128, 8, H], F32)     # [c%128, c//128, g]
            nc.sync.dma_start(
                waqT_sb[:], waqT_d.rearrange("(kc p) g -> p kc g", p=128)
            )
            hc_sb = pre.tile([128, 2, H], F32)       # [b, which, h]
            nc.sync.dma_start(hc_sb[:, 0, :], ht_d)
            nc.sync.dma_start(hc_sb[:, 1, :], ct_d)

            qT_sb = pre.tile([128, 8, BS], F32)      # [c%128, c//128, b]
            for t in range(2):
                psT = pre_ps.tile([128, 512], F32)
                for jj in range(4):
                    nc.tensor.transpose(
                        psT[:, jj * 128 : (jj + 1) * 128],
                        hc_sb[:, t, jj * 128 : (jj + 1) * 128],
                        ident[:],
                    )
                nc.vector.tensor_copy(
                    qT_sb[:, 4 * t : 4 * (t + 1), :].rearrange("p a b -> p (a b)"),
                    psT[:],
                )
            for m in range(4):
                psQ = pre_ps.tile([128, BS], F32)
                for k in range(8):
                    nc.tensor.matmul(
                        psQ[:],
                        waqT_sb[:, k, m * 128 : (m + 1) * 128],
                        qT_sb[:, k, :],
                        start=(k == 0),
                        stop=(k == 7),
                    )
                nc.scalar.add(qb_sb[:, m, :], psQ[:], ba_sb[:, m : m + 1])

        # ---- main pools ----
        natp = ctx.enter_context(tc.tile_pool(name="nat", bufs=NAT_BUFS))
        hhTp = ctx.enter_context(tc.tile_pool(name="hhT", bufs=2))
        scorep = ctx.enter_context(tc.tile_pool(name="score", bufs=2))
        aup = ctx.enter_context(tc.tile_pool(name="au", bufs=2))
        alphap = ctx.enter_context(tc.tile_pool(name="alpha", bufs=2))
        alphaTp = ctx.enter_context(tc.tile_pool(name="alphaT", bufs=2))
        ep = ctx.enter_context(tc.tile_pool(name="esb", bufs=2))
        smallp = ctx.enter_context(tc.tile_pool(name="small", bufs=8))

        psTp = ctx.enter_context(tc.tile_pool(name="psT", bufs=2, space="PSUM"))
        psAp = ctx.enter_context(tc.tile_pool(name="psA", bufs=2, space="PSUM"))
        psVp = ctx.enter_context(tc.tile_pool(name="psV", bufs=1, space="PSUM"))
        psCp = ctx.enter_context(tc.tile_pool(name="psC", bufs=2, space="PSUM"))
        psATp = ctx.enter_context(tc.tile_pool(name="psAT", bufs=1, space="PSUM"))

        for g in range(BS // GROUP):
            au = aup.tile([GROUP, 256], F32, tag="au")
            nat_tiles = []
            for pj in range(PPG):
                j = g * PPG + pj
                # load pair (2 rows) of h_history, rounded to f32r
                nat2 = natp.tile([128, 2, 2, 512], F32R)   # [d%128, b2, d//128, h]
                nc.gpsimd.dma_start(
                    nat2[:],
                    hh_d[2 * j : 2 * j + 2].rearrange("b (two p) h -> p b two h", p=128),
                )
                # transpose to [h%128, h//128, (b2, d)]
                hhT = hhTp.tile([128, 4, 512], F32R)
                for hc in range(4):
                    psT = psTp.tile([128, 512], F32R)
                    for b2 in range(2):
                        for two in range(2):
                            nc.tensor.transpose(
                                psT[:, b2 * 256 + two * 128 : b2 * 256 + (two + 1) * 128],
                                nat2[:, b2, two, hc * 128 : (hc + 1) * 128],
                                ident_r[:],
                            )
                    nc.vector.tensor_copy(hhT[:, hc, :], psT[:])
                # score projection + tanh
                score = scorep.tile([128, 4, 512], F32R)   # [g%128, g//128, (b2, d)]
                for m in range(4):
                    psA = psAp.tile([128, 512], F32)
                    for k in range(4):
                        nc.tensor.matmul(
                            psA[:],
                            wahT_sb[:, k, m * 128 : (m + 1) * 128],
                            hhT[:, k, :],
                            start=(k == 0),
                            stop=(k == 3),
                        )
                    for b2 in range(2):
                        bglob = 2 * j + b2
                        nc.scalar.activation(
                            score[:, m, b2 * 256 : (b2 + 1) * 256],
                            psA[:, b2 * 256 : (b2 + 1) * 256],
                            AF.Tanh,
                            bias=qb_sb[:, m, bglob : bglob + 1],
                            scale=1.0,
                        )
                # v-dot -> au rows
                psV = psVp.tile([1, 512], F32)
                for k in range(4):
                    nc.tensor.matmul(
                        psV[:],
                        v_r[:, k : k + 1],
                        score[:, k, :],
                        start=(k == 0),
                        stop=(k == 3),
                    )
                au_pair = aup.tile([1, 2, 256], F32, tag="au_pair")
                nc.vector.tensor_copy(
                    au_pair[:].rearrange("one a d -> one (a d)"), psV[0:1, :]
                )
                nc.sync.dma_start(au_stage[j : j + 1, :, :], au_pair[:])
                nc.sync.dma_start(au[2 * pj : 2 * pj + 2, :], au_stage[j])
                nat_tiles.append(nat2)

            # softmax over d (free dim), batched over GROUP rows
            nm = smallp.tile([GROUP, 1], F32)
            nc.vector.reduce_max(nm[:], au[:], axis=AX.X, negate=True)
            ex = alphap.tile([GROUP, 256], F32, tag="ex")
            sm = smallp.tile([GROUP, 1], F32)
            nc.scalar.activation(
                ex[:], au[:], AF.Exp, bias=nm[:], scale=1.0, accum_out=sm[:]
            )
            rc = smallp.tile([GROUP, 1], F32)
            nc.vector.reciprocal(rc[:], sm[:])
            alpha = alphap.tile([GROUP, 256], F32R, tag="alpha")
            nc.vector.tensor_scalar_mul(alpha[:], ex[:], rc[:])
            nc.sync.dma_start(
                alpha_out[g * GROUP : (g + 1) * GROUP, :], alpha[:].bitcast(F32)
            )

            # alpha transpose -> [d%128, d//128, gj]
            psAT = psATp.tile([128, 2, GROUP], F32R)
            for dc in range(2):
                nc.tensor.transpose(
                    psAT[:, dc, :],
                    alpha[:, dc * 128 : (dc + 1) * 128],
                    ident_r[:GROUP, :GROUP],
                )
            alphaT = alphaTp.tile([128, 2, GROUP], F32R)
            nc.vector.tensor_copy(alphaT[:], psAT[:])

            # weighted sum e_t = alpha @ h_history
            for pj in range(PPG):
                nat2 = nat_tiles[pj]
                e_pair = ep.tile([1, 2, 512], F32)
                for b2 in range(2):
                    gj = 2 * pj + b2
                    psC = psCp.tile([1, 512], F32)
                    for two in range(2):
                        nc.tensor.matmul(
                            psC[:],
                            alphaT[:, two, gj : gj + 1],
                            nat2[:, b2, two, :],
                            start=(two == 0),
                            stop=(two == 1),
                        )
                    nc.scalar.copy(e_pair[0:1, b2, :], psC[0:1, :])
                j = g * PPG + pj
                nc.sync.dma_start(e_view[j : j + 1, :, :], e_pair[:])

    nc.compile()
    return nc


def _get_nc():
    if "nc" not in _NC_CACHE:
        _NC_CACHE["nc"] = build_nc()
    return _NC_CACHE["nc"]


def run(in_maps, **kwargs):
    nc = _get_nc()
    return run_bass_kernel_spmd(nc, in_maps, core_ids=list(range(NCORES)), **kwargs)


def make_in_maps(h_tilde, c_t, h_history, Waq, Wah, ba, v_t):
    h_tilde = np.ascontiguousarray(np.asarray(h_tilde, dtype=np.float32))
    c_t = np.ascontiguousarray(np.asarray(c_t, dtype=np.float32))
    h_history = np.ascontiguousarray(np.asarray(h_history, dtype=np.float32))
    waqT = np.ascontiguousarray(np.asarray(Waq, dtype=np.float32).T)
    wahT = np.ascontiguousarray(np.asarray(Wah, dtype=np.float32).T)
    ba = np.ascontiguousarray(np.asarray(ba, dtype=np.float32))
    v_t = np.ascontiguousarray(np.asarray(v_t, dtype=np.float32))
    in_maps = []
    for i in range(NCORES):
        s = slice(i * BS, (i + 1) * BS)
        in_maps.append(
            {
                "h_tilde": h_tilde[s],
                "c_t": c_t[s],
                "h_history": h_history[s],
                "WaqT": waqT,
                "WahT": wahT,
                "ba": ba,
                "v_t": v_t,
            }
        )
    return in_maps


def kernel(h_tilde, c_t, h_history, Waq, Wah, ba, v_t):
    in_maps = make_in_maps(h_tilde, c_t, h_history, Waq, Wah, ba, v_t)
    res = run(in_maps)
    e_t = np.concatenate([r["e_t"] for r in res.results], axis=0)
    alpha = np.concatenate([r["alpha"] for r in res.results], axis=0)
    return e_t, alpha


# revision 12
# speedup vs baseline: 1.2999x; 1.2999x over previous
"""CurrentHistoricalAttention Trainium2 kernel (8 NeuronCores, data-parallel over B).

Math (per batch row b):
    q = [h_tilde; c_t] @ Waq.T + ba          [H]
    score = tanh(q[None, :] + h_history[b] @ Wah.T)   [D, H]
    au = score @ v_t                          [D]
    alpha = softmax(au)                       [D]
    e_t = alpha @ h_history[b]                [H]
Returns (e_t [B, H], alpha [B, D]).

Design (per core, 128 rows of B):
  - h_history tiles are loaded once, as f32r (rounded fp32, 1 cyc/row on PE),
    in natural [d, h] layout; PE-transposed on-chip to [h, d] for the
    score projection (contraction over h); the natural copy feeds the final
    alpha-weighted sum (contraction over d).
  - score projection: lhsT = Wah.T chunks (stationary), rhs = transposed
    h_history, accumulate over 4 k-chunks in PSUM.
  - tanh + (q+ba) bias fused into one ScalarE activation per [128, 256] tile.
  - v-dot as M=1 f32r matmuls; softmax batched over groups of 16 rows;
    alpha transposed via PE for the weighted-sum matmuls.
"""
import numpy as np
from contextlib import ExitStack

H = 512
B = 1024
D = 256
NCORES = 8
BS = B // NCORES        # 128 rows per core
GROUP = 16              # rows per softmax group
PPG = GROUP // 2        # pairs per group
NPAIR = BS // 2         # 64 pairs per core
NAT_BUFS = 14

import concourse.bass as bass
import concourse.mybir as mybir
import concourse.tile as tile
from concourse import bacc
from concourse.bass_utils import run_bass_kernel_spmd
from concourse.masks import make_identity

F32 = mybir.dt.float32
F32R = mybir.dt.float32r
AF = mybir.ActivationFunctionType
AX = mybir.AxisListType

_NC_CACHE = {}


def build_nc():
    nc = bacc.Bacc("TRN2", target_bir_lowering=False, debug=False)

    ht_d = nc.dram_tensor("h_tilde", [BS, H], F32, kind="ExternalInput").ap()
    ct_d = nc.dram_tensor("c_t", [BS, H], F32, kind="ExternalInput").ap()
    hh_d = nc.dram_tensor("h_history", [BS, D, H], F32, kind="ExternalInput").ap()
    waqT_d = nc.dram_tensor("WaqT", [2 * H, H], F32, kind="ExternalInput").ap()
    wahT_d = nc.dram_tensor("WahT", [H, H], F32, kind="ExternalInput").ap()
    ba_d = nc.dram_tensor("ba", [H], F32, kind="ExternalInput").ap()
    v_d = nc.dram_tensor("v_t", [H], F32, kind="ExternalInput").ap()

    e_out = nc.dram_tensor("e_t", [BS, H], F32, kind="ExternalOutput").ap()
    alpha_out = nc.dram_tensor("alpha", [BS, D], F32, kind="ExternalOutput").ap()
    au_stage = nc.dram_tensor("au_stage", [NPAIR, 2, 256], F32, kind="Internal").ap()
    e_view = e_out.rearrange("(j a) d -> j a d", a=2)

    with tile.TileContext(nc) as tc, ExitStack() as ctx:
        # ---- persistent constants ----
        const = ctx.enter_context(tc.tile_pool(name="const", bufs=1))
        ident = const.tile([128, 128], F32)
        make_identity(nc, ident[:])
        ident_r = const.tile([128, 128], F32R)
        nc.vector.tensor_copy(ident_r[:], ident[:])

        wahT_sb = const.tile([128, 4, H], F32R)     # [h%128, h//128, g]
        nc.gpsimd.dma_start(
            wahT_sb[:], wahT_d.rearrange("(kc p) g -> p kc g", p=128)
        )
        v_sb = const.tile([128, 4], F32)
        nc.sync.dma_start(v_sb[:], v_d.rearrange("(k p) -> p k", p=128))
        v_r = const.tile([128, 4], F32R)
        nc.vector.tensor_copy(v_r[:], v_sb[:])
        ba_sb = const.tile([128, 4], F32)
        nc.sync.dma_start(ba_sb[:], ba_d.rearrange("(m p) -> p m", p=128))

        qb_sb = const.tile([128, 4, BS], F32)        # [g%128, g//128, b]

        # ---- preamble: q projection (transposed) ----
        with ExitStack() as pctx:
            pre = pctx.enter_context(tc.tile_pool(name="pre", bufs=1))
            pre_ps = pctx.enter_context(tc.tile_pool(name="pre_ps", bufs=2, space="PSUM"))

            waqT_sb = pre.tile([128, 8, H], F32)     # [c%128, c//128, g]
            nc.sync.dma_start(
                waqT_sb[:], waqT_d.rearrange("(kc p) g -> p kc g", p=128)
            )
            hc_sb = pre.tile([128, 2, H], F32)       # [b, which, h]
            nc.sync.dma_start(hc_sb[:, 0, :], ht_d)
            nc.sync.dma_start(hc_sb[:, 1, :], ct_d)

            qT_sb = pre.tile([128, 8, BS], F32)      # [c%128, c//128, b]
            for t in range(2):
                psT = pre_ps.tile([128, 512], F32)
                for jj in range(4):
                    nc.tensor.transpose(
                        psT[:, jj * 128 : (jj + 1) * 128],
                        hc_sb[:, t, jj * 128 : (jj + 1) * 128],
                        ident[:],
                    )
                nc.vector.tensor_copy(
                    qT_sb[:, 4 * t : 4 * (t + 1), :].rearrange("p a b -> p (a b)"),
                    psT[:],
                )
            for m in range(4):
                psQ = pre_ps.tile([128, BS], F32)
                for k in range(8):
                    nc.tensor.matmul(
                        psQ[:],
                        waqT_sb[:, k, m * 128 : (m + 1) * 128],
                        qT_sb[:, k, :],
                        start=(k == 0),
                        stop=(k == 7),
                    )
                nc.scalar.add(qb_sb[:, m, :], psQ[:], ba_sb[:, m : m + 1])

        # ---- main pools ----
        natp = ctx.enter_context(tc.tile_pool(name="nat", bufs=NAT_BUFS))
        hhTp = ctx.enter_context(tc.tile_pool(name="hhT", bufs=2))
        scorep = ctx.enter_context(tc.tile_pool(name="score", bufs=2))
        aup = ctx.enter_context(tc.tile_pool(name="au", bufs=2))
        alphap = ctx.enter_context(tc.tile_pool(name="alpha", bufs=2))
        alphaTp = ctx.enter_context(tc.tile_pool(name="alphaT", bufs=2))
        ep = ctx.enter_context(tc.tile_pool(name="esb", bufs=2))
        smallp = ctx.enter_context(tc.tile_pool(name="small", bufs=8))

        psTp = ctx.enter_context(tc.tile_pool(name="psT", bufs=2, space="PSUM"))
        psAp = ctx.enter_context(tc.tile_pool(name="psA", bufs=2, space="PSUM"))
        psVp = ctx.enter_context(tc.tile_pool(name="psV", bufs=1, space="PSUM"))
        psCp = ctx.enter_context(tc.tile_pool(name="psC", bufs=2, space="PSUM"))
        psATp = ctx.enter_context(tc.tile_pool(name="psAT", bufs=1, space="PSUM"))

        for g in range(BS // GROUP):
            au = aup.tile([GROUP, 256], F32, tag="au")
            nat_tiles = []
            for pj in range(PPG):
                j = g * PPG + pj
                # load pair (2 rows) of h_history, rounded to f32r
                nat2 = natp.tile([128, 2, 2, 512], F32R)   # [d%128, b2, d//128, h]
                nc.gpsimd.dma_start(
                    nat2[:],
                    hh_d[2 * j : 2 * j + 2].rearrange("b (two p) h -> p b two h", p=128),
                )
                # transpose to [h%128, h//128, (b2, d)]
                hhT = hhTp.tile([128, 4, 512], F32R)
                for hc in range(4):
                    psT = psTp.tile([128, 512], F32R)
                    for b2 in range(2):
                        for two in range(2):
                            nc.tensor.transpose(
                                psT[:, b2 * 256 + two * 128 : b2 * 256 + (two + 1) * 128],
                                nat2[:, b2, two, hc * 128 : (hc + 1) * 128],
                                ident_r[:],
                            )
                    nc.vector.tensor_copy(hhT[:, hc, :], psT[:])
                # score projection + tanh
                score = scorep.tile([128, 4, 512], F32R)   # [g%128, g//128, (b2, d)]
                for m in range(4):
                    psA = psAp.tile([128, 512], F32)
                    for k in range(4):
                        nc.tensor.matmul(
                            psA[:],
                            wahT_sb[:, k, m * 128 : (m + 1) * 128],
                            hhT[:, k, :],
                            start=(k == 0),
                            stop=(k == 3),
                        )
                    for b2 in range(2):
                        bglob = 2 * j + b2
                        nc.scalar.activation(
                            score[:, m, b2 * 256 : (b2 + 1) * 256],
                            psA[:, b2 * 256 : (b2 + 1) * 256],
                            AF.Tanh,
                            bias=qb_sb[:, m, bglob : bglob + 1],
                            scale=1.0,
                        )
                # v-dot -> au rows
                psV = psVp.tile([1, 512], F32)
                for k in range(4):
                    nc.tensor.matmul(
                        psV[:],
                        v_r[:, k : k + 1],
                        score[:, k, :],
                        start=(k == 0),
                        stop=(k == 3),
                    )
                au_pair = aup.tile([1, 2, 256], F32, tag="au_pair")
                nc.vector.tensor_copy(
                    au_pair[:].rearrange("one a d -> one (a d)"), psV[0:1, :]
                )
                nc.sync.dma_start(au[2 * pj : 2 * pj + 2, :], au_pair[:])
                nat_tiles.append(nat2)

            # softmax over d (free dim), batched over GROUP rows
            nm = smallp.tile([GROUP, 1], F32)
            nc.vector.reduce_max(nm[:], au[:], axis=AX.X, negate=True)
            ex = alphap.tile([GROUP, 256], F32, tag="ex")
            sm = smallp.tile([GROUP, 1], F32)
            nc.scalar.activation(
                ex[:], au[:], AF.Exp, bias=nm[:], scale=1.0, accum_out=sm[:]
            )
            rc = smallp.tile([GROUP, 1], F32)
            nc.vector.reciprocal(rc[:], sm[:])
            alpha = alphap.tile([GROUP, 256], F32R, tag="alpha")
            nc.vector.tensor_scalar_mul(alpha[:], ex[:], rc[:])
            nc.sync.dma_start(
                alpha_out[g * GROUP : (g + 1) * GROUP, :], alpha[:].bitcast(F32)
            )

            # alpha transpose -> [d%128, d//128, gj]
            psAT = psATp.tile([128, 2, GROUP], F32R)
            for dc in range(2):
                nc.tensor.transpose(
                    psAT[:, dc, :],
                    alpha[:, dc * 128 : (dc + 1) * 128],
                    ident_r[:GROUP, :GROUP],
                )
            alphaT = alphaTp.tile([128, 2, GROUP], F32R)
            nc.vector.tensor_copy(alphaT[:], psAT[:])

            # weighted sum e_t = alpha @ h_history
            for pj in range(PPG):
                nat2 = nat_tiles[pj]
                e_pair = ep.tile([1, 2, 512], F32)
                for b2 in range(2):
                    gj = 2 * pj + b2
                    psC = psCp.tile([1, 512], F32)
                    for two in range(2):
                        nc.tensor.matmul(
                            psC[:],
                            alphaT[:, two, gj : gj + 1],
                            nat2[:, b2, two, :],
                            start=(two == 0),
                            stop=(two == 1),
                        )
                    nc.scalar.copy(e_pair[0:1, b2, :], psC[0:1, :])
                j = g * PPG + pj
                nc.sync.dma_start(e_view[j : j + 1, :, :], e_pair[:])

    nc.compile()
    return nc


def _get_nc():
    if "nc" not in _NC_CACHE:
        _NC_CACHE["nc"] = build_nc()
    return _NC_CACHE["nc"]


def run(in_maps, **kwargs):
    nc = _get_nc()
    return run_bass_kernel_spmd(nc, in_maps, core_ids=list(range(NCORES)), **kwargs)


def make_in_maps(h_tilde, c_t, h_history, Waq, Wah, ba, v_t):
    h_tilde = np.ascontiguousarray(np.asarray(h_tilde, dtype=np.float32))
    c_t = np.ascontiguousarray(np.asarray(c_t, dtype=np.float32))
    h_history = np.ascontiguousarray(np.asarray(h_history, dtype=np.float32))
    waqT = np.ascontiguousarray(np.asarray(Waq, dtype=np.float32).T)
    wahT = np.ascontiguousarray(np.asarray(Wah, dtype=np.float32).T)
    ba = np.ascontiguousarray(np.asarray(ba, dtype=np.float32))
    v_t = np.ascontiguousarray(np.asarray(v_t, dtype=np.float32))
    in_maps = []
    for i in range(NCORES):
        s = slice(i * BS, (i + 1) * BS)
        in_maps.append(
            {
                "h_tilde": h_tilde[s],
                "c_t": c_t[s],
                "h_history": h_history[s],
                "WaqT": waqT,
                "WahT": wahT,
                "ba": ba,
                "v_t": v_t,
            }
        )
    return in_maps


def kernel(h_tilde, c_t, h_history, Waq, Wah, ba, v_t):
    in_maps = make_in_maps(h_tilde, c_t, h_history, Waq, Wah, ba, v_t)
    res = run(in_maps)
    e_t = np.concatenate([r["e_t"] for r in res.results], axis=0)
    alpha = np.concatenate([r["alpha"] for r in res.results], axis=0)
    return e_t, alpha
